# revision 10
# baseline (speedup 1.0000x reference)
"""Trainium2 Bass kernel for nn_Net_50440095924334 (retrieval_knn).

Pipeline (per reference):
  feats = x @ W_fc + b_fc                      [2048, 512]
  centers = segment_sum(feats[:1024], labels)  [4, 512]
  target_labels = argmin_c ||tgt - center_c||^2
  adj = (label_i == label_j), src-src block forced to I
  A_norm = D^-1/2 (adj + I) D^-1/2             [2048, 2048]
  h = relu(A_norm @ relu(A_norm @ (feats@W_g1) + b_g1) @ W_g2 + b_g2)
  returns (h, A_norm, feats)

Distribution (8 cores, uniform SPMD program; all per-core variation comes
through input data):
  - FC matmul: K-dim (25088) sharded 8 ways, bf16 inputs, fp32 PSUM accum.
    Partial feats are ReduceScattered per node-half, so core i owns source
    nodes [128i, 128i+128) and target nodes [1024+128i, 1024+128i+128) at
    fixed SBUF addresses; row types (source/target) are compile-time.
    The source-half RS and the tiny centroid AllReduce fire at the matmul
    midpoint and overlap the second half.
  - adjacency is rank-4: adj = onehot @ onehot.T (with src-src block = I),
    so A_norm rows and all GCN aggregations collapse to [4, d] sums:
      A_norm @ X = dinv * (c * Y + onehot @ G),  Y = dinv * X,
      G[c] = dinv_class[c] * U[c],  U[c] = sum_{j in class c} X_j,
      c = 2 for source rows (identity block + self loop), 1 for target rows.
    U is AllReduced per (src, tgt) half; dinv depends only on class counts,
    so the one-hot AllGather and the U AllReduce are issued back-to-back.
  - Each core computes/writes only its 256 rows of A_norm / h / feats.
"""

import numpy as np
import ml_dtypes

N = 2048
S = 1024
C = 4
FEAT_IN = 25088
D_FC = 512
D_G1 = 512
D_G2 = 256
NCORES = 8
KPC = 25          # padded 128-row k-chunks per core (4 cores have 24 real)
KSPLIT = [25, 25, 25, 25, 24, 24, 24, 24]
NT = 16           # node tiles of 128
OWN_T = 2         # node tiles owned per core: [0]=source half, [1]=target half

BF16 = ml_dtypes.bfloat16

_PROGRAM_CACHE = {}


def _build_program(repeat=1, single_core=False):
    key = ("nc", repeat, single_core)
    if key in _PROGRAM_CACHE:
        return _PROGRAM_CACHE[key]

    import concourse.bacc as bacc
    import concourse.tile as tile
    import concourse.mybir as mybir
    from concourse.masks import make_identity

    dt = mybir.dt
    f32 = dt.float32
    bf16 = dt.bfloat16
    Alu = mybir.AluOpType

    nc = bacc.Bacc("TRN2", target_bir_lowering=False, debug=False,
                   num_devices=(1 if single_core else NCORES))

    # ---------------- DRAM I/O ----------------
    # xt blocked: block (k, mg) = xT[k*128:(k+1)*128, mg*512:(mg+1)*512]
    xt = nc.dram_tensor("xt", [KPC * 4 * 128, 512], bf16, kind="ExternalInput").ap()
    wfc = nc.dram_tensor("wfc", [KPC * 128, D_FC], bf16, kind="ExternalInput").ap()
    oh_src = nc.dram_tensor("oh_src", [S, C], bf16, kind="ExternalInput").ap()
    o_own_src = nc.dram_tensor("o_own_src", [128, C], f32, kind="ExternalInput").ap()
    e_scaled = nc.dram_tensor("e_scaled", [OWN_T * 128, N], bf16, kind="ExternalInput").ap()
    bfc_row = nc.dram_tensor("bfc_row", [1, D_FC], f32, kind="ExternalInput").ap()
    bg1_row = nc.dram_tensor("bg1_row", [1, D_G1], f32, kind="ExternalInput").ap()
    bg2_row = nc.dram_tensor("bg2_row", [1, D_G2], f32, kind="ExternalInput").ap()
    bctr_in = nc.dram_tensor("bctr", [128, 4 * C], f32, kind="ExternalInput").ap()
    wg1_in = nc.dram_tensor("wg1", [D_G1, D_G1], bf16, kind="ExternalInput").ap()
    wg2_in = nc.dram_tensor("wg2", [D_G1, D_G2], bf16, kind="ExternalInput").ap()

    feats_sh = nc.dram_tensor("feats_sh", [OWN_T * 128, D_FC], f32, kind="ExternalOutput").ap()
    a_sh = nc.dram_tensor("a_sh", [OWN_T * 128, N], f32, kind="ExternalOutput").ap()
    h_sh = nc.dram_tensor("h_sh", [OWN_T * 128, D_G2], f32, kind="ExternalOutput").ap()

    RG = [list(range(NCORES))]
    AS = "Local" if single_core else "Shared"

    with tile.TileContext(nc) as tc:
        with (
            tc.tile_pool(name="dram", bufs=1, space="DRAM") as dram,
            tc.tile_pool(name="persist", bufs=1) as pp,
            tc.tile_pool(name="small", bufs=1) as sp,
            tc.tile_pool(name="ps_big", bufs=4, space="PSUM") as ps_big,
        ):
            for _rep in range(repeat):
                # collective bounce buffers (DRAM pool tiles => dep tracking)
                R = _rep
                pf_b = [dram.tile([S, D_FC], bf16, name=f"pf_b{hf}_{R}", tag=f"pfb{hf}_{R}")
                        for hf in range(2)]
                fo_d = [dram.tile([128, D_FC], bf16, name=f"fo_d{hf}_{R}", tag=f"fod{hf}_{R}")
                        for hf in range(2)]
                ct_bounce = dram.tile([128, 4 * C], f32, name=f"ct_bounce{R}", tag=f"ctb{R}")
                ct_ar = dram.tile([128, 4 * C], f32, addr_space=AS, name=f"ct_ar{R}", tag=f"cta{R}")
                og_in = dram.tile([OWN_T * 128, C], f32, name=f"og_in{R}", tag=f"ogi{R}")
                og_out = dram.tile([N, C], f32, addr_space=AS, name=f"og_out{R}", tag=f"ogo{R}")
                u1_in = dram.tile([C, 2 * D_G1], f32, name=f"u1_in{R}", tag=f"u1i{R}")
                u1_ar = dram.tile([C, 2 * D_G1], f32, addr_space=AS, name=f"u1_ar{R}", tag=f"u1a{R}")
                g2_in = dram.tile([C, 2 * D_G2], f32, name=f"g2_in{R}", tag=f"g2i{R}")
                g2_ar = dram.tile([C, 2 * D_G2], f32, addr_space=AS, name=f"g2_ar{R}", tag=f"g2a{R}")

                # ---------------- constants ----------------
                eye = pp.tile([128, 128], f32, tag="eye")
                make_identity(nc, eye[:])
                ones_col = pp.tile([128, 1], f32, tag="ones_col")
                nc.vector.memset(ones_col[:], 1.0)
                ones_row = pp.tile([1, 128], f32, tag="ones_row")
                nc.vector.memset(ones_row[:], 1.0)

                oh_t = [pp.tile([128, C], bf16, tag=f"oh_{t}", name=f"oh_{t}") for t in range(8)]
                for t in range(8):
                    nc.sync.dma_start(oh_t[t][:], oh_src[t * 128:(t + 1) * 128, :])
                oso = pp.tile([128, C], f32, tag="oso")
                nc.sync.dma_start(oso[:], o_own_src[:])
                bctr = pp.tile([128, 4 * C], f32, tag="bctr")
                nc.sync.dma_start(bctr[:], bctr_in[:])

                # ---------------- phase 1: big FC matmul (bf16, K-sharded) --
                ct_sb = pp.tile([128, 4 * C], f32, tag="ct_sb")
                with (
                    tc.tile_pool(name="xt_pool", bufs=12) as xt_pool,
                    tc.tile_pool(name="w_pool", bufs=KPC) as w_pool,
                    tc.tile_pool(name="pf_pool", bufs=4) as pf_pool,
                    tc.tile_pool(name="ps_ct", bufs=1, space="PSUM") as ps_ct,
                ):
                    ps_c = ps_ct.tile([128, 4 * C], f32, tag="ps_c")
                    w_t = []
                    for k in range(KPC):
                        wk = w_pool.tile([128, D_FC], bf16, tag="w", name=f"w_{k}")
                        nc.sync.dma_start(wk[:], wfc[k * 128:(k + 1) * 128, :])
                        w_t.append(wk)

                    for mg in range(4):
                        pss = [ps_big.tile([128, D_FC], f32, tag="ps_mm", name=f"ps_mm{mi}")
                               for mi in range(4)]
                        for k in range(KPC):
                            xtb = xt_pool.tile([128, 512], bf16, tag="xt", name="xtb")
                            nc.sync.dma_start(xtb[:], xt[(k * 4 + mg) * 128:(k * 4 + mg + 1) * 128, :])
                            for mi in range(4):
                                nc.tensor.matmul(
                                    pss[mi][:], xtb[:, mi * 128:(mi + 1) * 128], w_t[k][:],
                                    start=(k == 0), stop=(k == KPC - 1))
                        for mi in range(4):
                            m = mg * 4 + mi
                            hf, mh = (0, m) if m < 8 else (1, m - 8)
                            pf = pf_pool.tile([128, D_FC], bf16, tag="pf", name="pf")
                            nc.vector.tensor_copy(pf[:], pss[mi][:])
                            nc.sync.dma_start(pf_b[hf][mh * 128:(mh + 1) * 128, :], pf[:])
                            if m < 8:
                                # partial centersT (bf16 lhsT; exact one-hot rhs)
                                for fc in range(4):
                                    nc.tensor.matmul(
                                        ps_c[:, fc * C:(fc + 1) * C],
                                        pf[:, fc * 128:(fc + 1) * 128], oh_t[m][:],
                                        start=(m == 0), stop=(m == 7))
                        if mg == 1:
                            # src half done: fire AR(centersT) + RS(src half)
                            nc.vector.tensor_copy(ct_sb[:], ps_c[:])
                            nc.sync.dma_start(ct_bounce[:], ct_sb[:])
                            if single_core:
                                nc.sync.dma_start(ct_ar[:], ct_bounce[:])
                                nc.sync.dma_start(fo_d[0][:], pf_b[0][0:128, :])
                            else:
                                nc.gpsimd.collective_compute(
                                    "AllReduce", Alu.add, replica_groups=RG,
                                    ins=[ct_bounce.opt()], outs=[ct_ar.opt()])
                                nc.gpsimd.collective_compute(
                                    "ReduceScatter", Alu.add, replica_groups=RG,
                                    ins=[pf_b[0].opt()], outs=[fo_d[0].opt()])
                    if single_core:
                        nc.sync.dma_start(fo_d[1][:], pf_b[1][0:128, :])
                    else:
                        nc.gpsimd.collective_compute(
                            "ReduceScatter", Alu.add, replica_groups=RG,
                            ins=[pf_b[1].opt()], outs=[fo_d[1].opt()])

                # ---------------- small persistent loads (overlap RS) -------
                brow_fc = pp.tile([1, D_FC], f32, tag="brow_fc")
                nc.sync.dma_start(brow_fc[:], bfc_row[:])
                brow_g1 = pp.tile([1, D_G1], f32, tag="brow_g1")
                nc.sync.dma_start(brow_g1[:], bg1_row[:])
                brow_g2 = pp.tile([1, D_G2], f32, tag="brow_g2")
                nc.sync.dma_start(brow_g2[:], bg2_row[:])

                def bcast_row(row, d, tag):
                    ps = ps_big.tile([128, d], f32, tag="ps_mm", name="ps_b")
                    nc.tensor.matmul(ps[:], ones_row[:], row[:], start=True, stop=True)
                    t = pp.tile([128, d], f32, tag=tag, name=tag)
                    nc.vector.tensor_copy(t[:], ps[:])
                    return t

                bb_fc = bcast_row(brow_fc, D_FC, "bb_fc")
                bb_g1 = bcast_row(brow_g1, D_G1, "bb_g1")
                bb_g2 = bcast_row(brow_g2, D_G2, "bb_g2")

                wg1 = [pp.tile([128, D_G1], bf16, tag=f"wg1_{k}", name=f"wg1_{k}") for k in range(4)]
                for k in range(4):
                    nc.sync.dma_start(wg1[k][:], wg1_in[k * 128:(k + 1) * 128, :])
                wg2 = [pp.tile([128, D_G2], bf16, tag=f"wg2_{k}", name=f"wg2_{k}") for k in range(4)]
                for k in range(4):
                    nc.sync.dma_start(wg2[k][:], wg2_in[k * 128:(k + 1) * 128, :])
                e_t = [pp.tile([128, N], bf16, tag=f"e_{t}", name=f"e_{t}") for t in range(OWN_T)]
                for t in range(OWN_T):
                    nc.sync.dma_start(e_t[t][:], e_scaled[t * 128:(t + 1) * 128, :])

                with (
                    tc.tile_pool(name="ps_tr", bufs=2, space="PSUM") as ps_tr,
                    tc.tile_pool(name="ps_wide", bufs=1, space="PSUM") as ps_wide,
                ):
                    # centersT back (+ bias correction: outer(b_fc, counts_src))
                    ct2 = pp.tile([128, 4 * C], f32, tag="ct2")
                    nc.sync.dma_start(ct2[:], ct_ar[:])
                    nc.vector.tensor_tensor(ct2[:], ct2[:], bctr[:], op=Alu.add)

                    # cnorm = -0.5 * sum_f center_c^2   -> [1, C]
                    sq = sp.tile([128, 4 * C], f32, tag="sq")
                    nc.vector.tensor_tensor(sq[:], ct2[:], ct2[:], op=Alu.mult)
                    ps_cn = ps_tr.tile([1, 4 * C], f32, tag="pst", name="ps_cn")
                    nc.tensor.matmul(ps_cn[:], ones_col[:], sq[:], start=True, stop=True)
                    cn16 = sp.tile([1, 4 * C], f32, tag="cn16")
                    nc.vector.tensor_copy(cn16[:], ps_cn[:])
                    cnA = sp.tile([1, C], f32, tag="cnA")
                    nc.vector.tensor_tensor(cnA[:], cn16[:, 0:C], cn16[:, C:2 * C], op=Alu.add)
                    cnB = sp.tile([1, C], f32, tag="cnB")
                    nc.vector.tensor_tensor(cnB[:], cn16[:, 2 * C:3 * C], cn16[:, 3 * C:4 * C], op=Alu.add)
                    neghcn = sp.tile([1, C], f32, tag="neghcn")
                    nc.vector.tensor_tensor(neghcn[:], cnA[:], cnB[:], op=Alu.add)
                    nc.vector.tensor_scalar(neghcn[:], neghcn[:], -0.5, None, op0=Alu.mult)

                    # feats_own (+bias): tile0 = src half rows, tile1 = tgt half
                    feats_own = [pp.tile([128, D_FC], f32, tag=f"feats_own_{t}",
                                         name=f"feats_own_{t}") for t in range(OWN_T)]
                    for t in range(OWN_T):
                        fbf = sp.tile([128, D_FC], bf16, tag="fbf", bufs=2, name="fbf")
                        nc.sync.dma_start(fbf[:], fo_d[t][:])
                        nc.vector.tensor_tensor(feats_own[t][:], fbf[:], bb_fc[:], op=Alu.add)
                        nc.sync.dma_start(feats_sh[t * 128:(t + 1) * 128, :], feats_own[t][:])

                    # transposed feats (bf16) for X1 matmuls / scores
                    ftT = [pp.tile([128, 128], bf16, tag=f"ftT_{i}", name=f"ftT_{i}")
                           for i in range(OWN_T * 4)]
                    for t in range(OWN_T):
                        for fc in range(4):
                            pst = ps_tr.tile([128, 128], f32, tag="pst", name="ps_t")
                            nc.tensor.transpose(pst[:], feats_own[t][:, fc * 128:(fc + 1) * 128], eye[:])
                            nc.vector.tensor_copy(ftT[t * 4 + fc][:], pst[:])

                    # bf16 copy of centersT for the scores matmul
                    ct2b = sp.tile([128, 4 * C], bf16, tag="ct2b")
                    nc.vector.tensor_copy(ct2b[:], ct2[:])

                    # scores -> target one-hot mask (tile 1 only)
                    ps_s = ps_tr.tile([128, C], f32, tag="pst", name="ps_s")
                    for fc in range(4):
                        nc.tensor.matmul(ps_s[:], ftT[4 + fc][:],
                                         ct2b[:, fc * C:(fc + 1) * C],
                                         start=(fc == 0), stop=False)
                    nc.tensor.matmul(ps_s[:], ones_row[:], neghcn[:], start=False, stop=True)
                    mx = sp.tile([128, 1], f32, tag="mx")
                    nc.vector.tensor_reduce(mx[:], ps_s[:], axis=mybir.AxisListType.X,
                                            op=Alu.max)
                    mask = pp.tile([128, C], f32, tag="mask")
                    nc.vector.tensor_scalar(mask[:], ps_s[:], mx[:], None, op0=Alu.is_ge)

                    o_own = [oso, mask]
                    for t in range(OWN_T):
                        nc.sync.dma_start(og_in[t * 128:(t + 1) * 128, :], o_own[t][:])

                    # X1 = feats_own @ W_g1 (unscaled; dinv applied post-AR)
                    x1_sb = []
                    for t in range(OWN_T):
                        ps_x = ps_big.tile([128, D_G1], f32, tag="ps_mm", name="ps_x1")
                        for fc in range(4):
                            nc.tensor.matmul(ps_x[:], ftT[t * 4 + fc][:], wg1[fc][:],
                                             start=(fc == 0), stop=(fc == 3))
                        xs = pp.tile([128, D_G1], f32, tag=f"x1_{t}", name=f"x1_{t}")
                        nc.vector.tensor_copy(xs[:], ps_x[:])
                        x1_sb.append(xs)

                    # U1 = [sum_src-class X1 | sum_tgt-class X1]  (no dinv yet)
                    ps_u1 = ps_wide.tile([C, 2 * D_G1], f32, tag="ps_g", name="ps_u1")
                    nc.tensor.matmul(ps_u1[:, 0:D_G1], oso[:], x1_sb[0][:],
                                     start=True, stop=True)
                    nc.tensor.matmul(ps_u1[:, D_G1:2 * D_G1], mask[:], x1_sb[1][:],
                                     start=True, stop=True)
                    u1_sb = sp.tile([C, 2 * D_G1], f32, tag="u1_sb")
                    nc.vector.tensor_copy(u1_sb[:], ps_u1[:])
                    nc.sync.dma_start(u1_in[:], u1_sb[:])

                    # collectives: AG(one-hots) then AR(U1), back-to-back
                    if single_core:
                        for _c in range(NCORES):
                            nc.sync.dma_start(og_out[_c * 256:(_c + 1) * 256, :], og_in[:])
                        nc.sync.dma_start(u1_ar[:], u1_in[:])
                    else:
                        nc.gpsimd.collective_compute(
                            "AllGather", Alu.bypass, replica_groups=RG,
                            ins=[og_in.opt()], outs=[og_out.opt()])
                        nc.gpsimd.collective_compute(
                            "AllReduce", Alu.add, replica_groups=RG,
                            ins=[u1_in.opt()], outs=[u1_ar.opt()])

                    # O_full tiles in global node order:
                    # src tile t -> og_out[256t : 256t+128]
                    # tgt tile t -> og_out[256(t-8)+128 : 256(t-8)+256]
                    o_full = [pp.tile([128, C], f32, tag=f"of_{t}", name=f"of_{t}")
                              for t in range(NT)]
                    for t in range(8):
                        nc.sync.dma_start(o_full[t][:], og_out[256 * t:256 * t + 128, :])
                    for t in range(8, NT):
                        b = 256 * (t - 8) + 128
                        nc.sync.dma_start(o_full[t][:], og_out[b:b + 128, :])

                    # counts / degrees / dinv (classes x {src,tgt})
                    ps_cnt = ps_tr.tile([C, 2], f32, tag="pst", name="ps_cnt")
                    for t in range(8):
                        nc.tensor.matmul(ps_cnt[:, 0:1], o_full[t][:], ones_col[:],
                                         start=(t == 0), stop=(t == 7))
                    for t in range(8, NT):
                        nc.tensor.matmul(ps_cnt[:, 1:2], o_full[t][:], ones_col[:],
                                         start=(t == 8), stop=(t == NT - 1))
                    cnt = sp.tile([C, 2], f32, tag="cnt")
                    nc.vector.tensor_copy(cnt[:], ps_cnt[:])

                    deg2 = sp.tile([C, 2], f32, tag="deg2")  # col0 src rows, col1 tgt
                    nc.vector.tensor_scalar(deg2[:, 0:1], cnt[:, 1:2], 2.0, None, op0=Alu.add)
                    nc.vector.tensor_tensor(deg2[:, 1:2], cnt[:, 0:1], cnt[:, 1:2], op=Alu.add)
                    nc.vector.tensor_scalar(deg2[:, 1:2], deg2[:, 1:2], 1.0, None, op0=Alu.add)

                    # dinv = rsqrt(deg): seed sqrt(1/deg) + 2 Newton steps
                    rec = sp.tile([C, 2], f32, tag="rec")
                    nc.vector.reciprocal(rec[:], deg2[:])
                    dinv2 = sp.tile([C, 2], f32, tag="dinv2")
                    nc.scalar.activation(dinv2[:], rec[:], mybir.ActivationFunctionType.Sqrt)
                    tmp_a = sp.tile([C, 2], f32, tag="tmp_a")
                    tmp_b = sp.tile([C, 2], f32, tag="tmp_b")
                    for _ in range(2):
                        nc.vector.tensor_tensor(tmp_a[:], dinv2[:], dinv2[:], op=Alu.mult)
                        nc.vector.tensor_tensor(tmp_a[:], tmp_a[:], deg2[:], op=Alu.mult)
                        nc.vector.tensor_scalar(tmp_b[:], tmp_a[:], -0.5, 1.5,
                                                op0=Alu.mult, op1=Alu.add)
                        nc.vector.tensor_tensor(dinv2[:], dinv2[:], tmp_b[:], op=Alu.mult)
                    dinvsq2 = sp.tile([C, 2], f32, tag="dinvsq2")
                    nc.vector.tensor_tensor(dinvsq2[:], dinv2[:], dinv2[:], op=Alu.mult)

                    # ---------------- P matrix [4, 2048] ----------------
                    p_sb = pp.tile([C, N], f32, tag="p_sb")
                    for t in range(NT):
                        ps_o = ps_tr.tile([C, 128], f32, tag="pst", name="ps_o")
                        nc.tensor.transpose(ps_o[:], o_full[t][:], eye[:])
                        otT = sp.tile([C, 128], f32, tag="otT", bufs=2)
                        nc.vector.tensor_copy(otT[:], ps_o[:])
                        col = 0 if t < 8 else 1
                        ps_d = ps_tr.tile([128, 1], f32, tag="pst", name="ps_d")
                        nc.tensor.matmul(ps_d[:], otT[:], dinv2[:, col:col + 1],
                                         start=True, stop=True)
                        q = sp.tile([128, C], f32, tag="q", bufs=2)
                        nc.vector.tensor_scalar(q[:], o_full[t][:], ps_d[:], None,
                                                op0=Alu.mult)
                        ps_q = ps_tr.tile([C, 128], f32, tag="pst", name="ps_q")
                        nc.tensor.transpose(ps_q[:], q[:], eye[:])
                        nc.vector.tensor_copy(p_sb[:, t * 128:(t + 1) * 128], ps_q[:])

                    # own-row quantities (tile0: src type col=0, tile1: tgt col=1)
                    otT_own, pown, dinv_own, dinvsq_own = [], [], [], []
                    for t in range(OWN_T):
                        ps_o = ps_tr.tile([C, 128], f32, tag="pst", name="ps_o2")
                        nc.tensor.transpose(ps_o[:], o_own[t][:], eye[:])
                        ot = pp.tile([C, 128], f32, tag=f"otT_own_{t}", name=f"otT_own_{t}")
                        nc.vector.tensor_copy(ot[:], ps_o[:])
                        otT_own.append(ot)

                        col = t  # tile0 -> src col, tile1 -> tgt col
                        ps_d = ps_tr.tile([128, 2], f32, tag="pst", name="ps_d2")
                        nc.tensor.matmul(ps_d[:, 0:1], ot[:], dinv2[:, col:col + 1],
                                         start=True, stop=True)
                        nc.tensor.matmul(ps_d[:, 1:2], ot[:], dinvsq2[:, col:col + 1],
                                         start=True, stop=True)
                        dv = pp.tile([128, 2], f32, tag=f"dinv_own_{t}", name=f"dinv_own_{t}")
                        nc.vector.tensor_copy(dv[:], ps_d[:])
                        dinv_own.append(dv[:, 0:1])
                        dinvsq_own.append(dv[:, 1:2])

                        q = sp.tile([128, C], f32, tag="q_own")
                        nc.vector.tensor_scalar(q[:], o_own[t][:], dv[:, 0:1], None,
                                                op0=Alu.mult)
                        ps_q = ps_tr.tile([C, 128], f32, tag="pst", name="ps_q2")
                        nc.tensor.transpose(ps_q[:], q[:], eye[:])
                        po = pp.tile([C, 128], f32, tag=f"pown_{t}", name=f"pown_{t}")
                        nc.vector.tensor_copy(po[:], ps_q[:])
                        pown.append(po)

                    # ---------------- A_norm rows ----------------
                    # tile0 (src rows): cols < 1024 diag-only; cols >= 1024 dense
                    for j in range(2):
                        a_sb = sp.tile([128, 512], f32, tag="a_sb", bufs=2)
                        nc.vector.tensor_scalar(a_sb[:], e_t[0][:, j * 512:(j + 1) * 512],
                                                dinvsq_own[0], None, op0=Alu.mult)
                        nc.sync.dma_start(a_sh[0:128, j * 512:(j + 1) * 512], a_sb[:])
                    for j in range(2, 4):
                        ps_a = ps_big.tile([128, 512], f32, tag="ps_mm", name="ps_a")
                        nc.tensor.matmul(ps_a[:], pown[0][:], p_sb[:, j * 512:(j + 1) * 512],
                                         start=True, stop=True)
                        a_sb = sp.tile([128, 512], f32, tag="a_sb", bufs=2)
                        nc.vector.scalar_tensor_tensor(
                            a_sb[:], e_t[0][:, j * 512:(j + 1) * 512], dinvsq_own[0],
                            ps_a[:], op0=Alu.mult, op1=Alu.add)
                        nc.sync.dma_start(a_sh[0:128, j * 512:(j + 1) * 512], a_sb[:])
                    for j in range(4):
                        ps_a = ps_big.tile([128, 512], f32, tag="ps_mm", name="ps_a2")
                        nc.tensor.matmul(ps_a[:], pown[1][:], p_sb[:, j * 512:(j + 1) * 512],
                                         start=True, stop=True)
                        a_sb = sp.tile([128, 512], f32, tag="a_sb", bufs=2)
                        nc.vector.scalar_tensor_tensor(
                            a_sb[:], e_t[1][:, j * 512:(j + 1) * 512], dinvsq_own[1],
                            ps_a[:], op0=Alu.mult, op1=Alu.add)
                        nc.sync.dma_start(a_sh[128:256, j * 512:(j + 1) * 512], a_sb[:])

                    # ---------------- GCN layer 1 (post-AR) ----------------
                    u1r = sp.tile([C, 2 * D_G1], f32, tag="u1r")
                    nc.sync.dma_start(u1r[:], u1_ar[:])
                    # G_src = dinv_src_class * U_src ; G_tgt = dinv_tgt_class * U_tgt
                    g1_tgt = sp.tile([C, D_G1], f32, tag="g1_tgt")
                    nc.vector.tensor_scalar(g1_tgt[:], u1r[:, D_G1:2 * D_G1],
                                            dinv2[:, 1:2], None, op0=Alu.mult)
                    g1_all = sp.tile([C, D_G1], f32, tag="g1_all")
                    nc.vector.scalar_tensor_tensor(g1_all[:], u1r[:, 0:D_G1],
                                                   dinv2[:, 0:1], g1_tgt[:],
                                                   op0=Alu.mult, op1=Alu.add)
                    g1_use = [g1_tgt, g1_all]
                    c_own = [2.0, 1.0]

                    h1 = []
                    for t in range(OWN_T):
                        yt = sp.tile([128, D_G1], f32, tag=f"y1_{t}", name=f"y1_{t}")
                        nc.vector.tensor_scalar(yt[:], x1_sb[t][:], dinv_own[t], None,
                                                op0=Alu.mult)
                        ps_h = ps_big.tile([128, D_G1], f32, tag="ps_mm", name="ps_h1")
                        nc.tensor.matmul(ps_h[:], otT_own[t][:], g1_use[t][:],
                                         start=True, stop=True)
                        u = sp.tile([128, D_G1], f32, tag="u1t", bufs=2)
                        nc.vector.scalar_tensor_tensor(u[:], yt[:], c_own[t], ps_h[:],
                                                       op0=Alu.mult, op1=Alu.add)
                        ht = pp.tile([128, D_G1], f32, tag=f"h1_{t}", name=f"h1_{t}")
                        nc.vector.scalar_tensor_tensor(ht[:], u[:], dinv_own[t], bb_g1[:],
                                                       op0=Alu.mult, op1=Alu.add)
                        nc.vector.tensor_scalar(ht[:], ht[:], 0.0, None, op0=Alu.max)
                        h1.append(ht)

                    # ---------------- GCN layer 2 ----------------
                    y2 = []
                    for t in range(OWN_T):
                        h1T = []
                        for fc in range(4):
                            pst = ps_tr.tile([128, 128], f32, tag="pst", name="ps_t2")
                            nc.tensor.transpose(pst[:], h1[t][:, fc * 128:(fc + 1) * 128], eye[:])
                            hT = sp.tile([128, 128], bf16, tag=f"h1T_{fc}", bufs=2,
                                         name=f"h1T_{fc}")
                            nc.vector.tensor_copy(hT[:], pst[:])
                            h1T.append(hT)
                        ps_x = ps_big.tile([128, D_G2], f32, tag="ps_mm", name="ps_x2")
                        for fc in range(4):
                            nc.tensor.matmul(ps_x[:], h1T[fc][:], wg2[fc][:],
                                             start=(fc == 0), stop=(fc == 3))
                        yt = pp.tile([128, D_G2], f32, tag=f"y2_{t}", name=f"y2_{t}")
                        nc.vector.tensor_scalar(yt[:], ps_x[:], dinv_own[t], None, op0=Alu.mult)
                        y2.append(yt)

                    ps_g2 = ps_wide.tile([C, 2 * D_G2], f32, tag="ps_g", name="ps_g2")
                    nc.tensor.matmul(ps_g2[:, 0:D_G2], oso[:], y2[0][:], start=True, stop=True)
                    nc.tensor.matmul(ps_g2[:, D_G2:2 * D_G2], mask[:], y2[1][:],
                                     start=True, stop=True)
                    g2_sb = sp.tile([C, 2 * D_G2], f32, tag="g2_sb")
                    nc.vector.tensor_copy(g2_sb[:], ps_g2[:])
                    nc.sync.dma_start(g2_in[:], g2_sb[:])
                    if single_core:
                        nc.sync.dma_start(g2_ar[:], g2_in[:])
                    else:
                        nc.gpsimd.collective_compute(
                            "AllReduce", Alu.add, replica_groups=RG,
                            ins=[g2_in.opt()], outs=[g2_ar.opt()])
                    g2r = sp.tile([C, 2 * D_G2], f32, tag="g2r")
                    nc.sync.dma_start(g2r[:], g2_ar[:])
                    g2_tgt = sp.tile([C, D_G2], f32, tag="g2_tgt")
                    nc.vector.tensor_copy(g2_tgt[:], g2r[:, D_G2:2 * D_G2])
                    g2_all = sp.tile([C, D_G2], f32, tag="g2_all")
                    nc.vector.tensor_tensor(g2_all[:], g2r[:, 0:D_G2], g2_tgt[:], op=Alu.add)
                    g2_use = [g2_tgt, g2_all]

                    for t in range(OWN_T):
                        ps_h = ps_big.tile([128, D_G2], f32, tag="ps_mm", name="ps_h2")
                        nc.tensor.matmul(ps_h[:], otT_own[t][:], g2_use[t][:],
                                         start=True, stop=True)
                        u = sp.tile([128, D_G2], f32, tag="u2t", bufs=2)
                        nc.vector.scalar_tensor_tensor(u[:], y2[t][:], c_own[t], ps_h[:],
                                                       op0=Alu.mult, op1=Alu.add)
                        hh = sp.tile([128, D_G2], f32, tag="hh", bufs=2)
                        nc.vector.scalar_tensor_tensor(hh[:], u[:], dinv_own[t], bb_g2[:],
                                                       op0=Alu.mult, op1=Alu.add)
                        nc.vector.tensor_scalar(hh[:], hh[:], 0.0, None, op0=Alu.max)
                        nc.sync.dma_start(h_sh[t * 128:(t + 1) * 128, :], hh[:])

    nc.compile()
    _PROGRAM_CACHE[key] = nc
    return nc


def _host_inputs(x, W_fc, b_fc, W_g1, b_g1, W_g2, b_g2, source_labels):
    """Build per-core in_maps."""
    x = np.asarray(x, dtype=np.float32)
    W_fc = np.asarray(W_fc, dtype=np.float32)
    b_fc = np.asarray(b_fc, dtype=np.float32)
    labels = np.asarray(source_labels).astype(np.int64)

    x_bf = x.astype(BF16)
    w_bf = W_fc.astype(BF16)
    wg1_bf = np.asarray(W_g1, np.float32).astype(BF16)
    wg2_bf = np.asarray(W_g2, np.float32).astype(BF16)

    onehot_src = np.zeros((S, C), np.float32)
    onehot_src[np.arange(S), labels] = 1.0
    counts_src = onehot_src.sum(axis=0)

    # bias correction for centers: centersT chunks bctr[128, 4*C]
    bctr = np.zeros((128, 4 * C), np.float32)
    for fc in range(4):
        bctr[:, fc * C:(fc + 1) * C] = np.outer(b_fc[fc * 128:(fc + 1) * 128],
                                                counts_src)

    offs = np.cumsum([0] + KSPLIT)
    in_maps = []
    r = np.arange(128)
    for i in range(NCORES):
        c0 = offs[i] * 128
        nk = offs[i + 1] - offs[i]
        c1 = offs[i + 1] * 128
        xt_i = np.zeros((KPC, 4, 128, 512), BF16)
        xT = x_bf[:, c0:c1].T  # [nk*128, 2048]
        xt_i[:nk] = xT.reshape(nk, 128, 4, 512).transpose(0, 2, 1, 3)
        xt_i = np.ascontiguousarray(xt_i.reshape(KPC * 4 * 128, 512))
        wf_i = np.zeros((KPC * 128, D_FC), BF16)
        wf_i[:nk * 128] = w_bf[c0:c1]

        # own rows: src nodes [128i, 128i+128), tgt nodes [1024+128i, ...)
        e_sc = np.zeros((256, N), BF16)
        e_sc[r, 128 * i + r] = BF16(2.0)
        e_sc[128 + r, 1024 + 128 * i + r] = BF16(1.0)

        in_maps.append({
            "xt": xt_i,
            "wfc": wf_i,
            "oh_src": onehot_src.astype(BF16),
            "o_own_src": np.ascontiguousarray(onehot_src[128 * i:128 * (i + 1)]),
            "e_scaled": e_sc,
            "bfc_row": b_fc.reshape(1, D_FC),
            "bg1_row": np.asarray(b_g1, np.float32).reshape(1, D_G1),
            "bg2_row": np.asarray(b_g2, np.float32).reshape(1, D_G2),
            "bctr": bctr,
            "wg1": wg1_bf,
            "wg2": wg2_bf,
        })
    return in_maps


def _assemble(parts):
    """parts[i]: [256, d] rows = (src nodes 128i.., tgt nodes 1024+128i..)."""
    d = parts[0].shape[1]
    full = np.empty((N, d), np.float32)
    for i in range(NCORES):
        full[128 * i:128 * (i + 1)] = parts[i][0:128]
        full[1024 + 128 * i:1024 + 128 * (i + 1)] = parts[i][128:256]
    return full


def kernel(x, W_fc, b_fc, W_g1, b_g1, W_g2, b_g2, source_labels, source_length):
    from concourse import bass_utils

    assert int(source_length) == S
    nc = _build_program()
    in_maps = _host_inputs(x, W_fc, b_fc, W_g1, b_g1, W_g2, b_g2, source_labels)
    res = bass_utils.run_bass_kernel_spmd(nc, in_maps, list(range(NCORES)))
    h = _assemble([np.asarray(res.results[i]["h_sh"]) for i in range(NCORES)])
    a = _assemble([np.asarray(res.results[i]["a_sh"]) for i in range(NCORES)])
    f = _assemble([np.asarray(res.results[i]["feats_sh"]) for i in range(NCORES)])
    return (h, a, f)


# revision 12
# speedup vs baseline: 3.0456x; 3.0456x over previous
"""Trainium2 Bass kernel for nn_Net_50440095924334 (retrieval_knn).

Pipeline (per reference):
  feats = x @ W_fc + b_fc                      [2048, 512]
  centers = segment_sum(feats[:1024], labels)  [4, 512]
  target_labels = argmin_c ||tgt - center_c||^2
  adj = (label_i == label_j), src-src block forced to I
  A_norm = D^-1/2 (adj + I) D^-1/2             [2048, 2048]
  h = relu(A_norm @ relu(A_norm @ (feats@W_g1) + b_g1) @ W_g2 + b_g2)
  returns (h, A_norm, feats)

Distribution (8 cores, uniform SPMD program; all per-core variation comes
through input data):
  - FC matmul: K-dim (25088) sharded 8 ways, bf16 inputs, fp32 PSUM accum.
    Partial feats are ReduceScattered per node-half, so core i owns source
    nodes [128i, 128i+128) and target nodes [1024+128i, 1024+128i+128) at
    fixed SBUF addresses; row types (source/target) are compile-time.
    The source-half RS and the tiny centroid AllReduce fire at the matmul
    midpoint and overlap the second half.
  - adjacency is rank-4: adj = onehot @ onehot.T (with src-src block = I),
    so A_norm rows and all GCN aggregations collapse to [4, d] sums:
      A_norm @ X = dinv * (c * Y + onehot @ G),  Y = dinv * X,
      G[c] = dinv_class[c] * U[c],  U[c] = sum_{j in class c} X_j,
      c = 2 for source rows (identity block + self loop), 1 for target rows.
    U is AllReduced per (src, tgt) half; dinv depends only on class counts,
    so the one-hot AllGather and the U AllReduce are issued back-to-back.
  - Each core computes/writes only its 256 rows of A_norm / h / feats.
"""

import numpy as np
import ml_dtypes

N = 2048
S = 1024
C = 4
FEAT_IN = 25088
D_FC = 512
D_G1 = 512
D_G2 = 256
NCORES = 8
KPC = 25          # padded 128-row k-chunks per core (4 cores have 24 real)
KSPLIT = [25, 25, 25, 25, 24, 24, 24, 24]
NT = 16           # node tiles of 128
OWN_T = 2         # node tiles owned per core: [0]=source half, [1]=target half

BF16 = ml_dtypes.bfloat16

_PROGRAM_CACHE = {}


def _build_program(repeat=1, single_core=False):
    key = ("nc", repeat, single_core)
    if key in _PROGRAM_CACHE:
        return _PROGRAM_CACHE[key]

    import concourse.bacc as bacc
    import concourse.tile as tile
    import concourse.mybir as mybir
    from concourse.masks import make_identity

    dt = mybir.dt
    f32 = dt.float32
    bf16 = dt.bfloat16
    Alu = mybir.AluOpType

    nc = bacc.Bacc("TRN2", target_bir_lowering=False, debug=False,
                   num_devices=(1 if single_core else NCORES))

    # ---------------- DRAM I/O ----------------
    # xt blocked: block (k, mg) = xT[k*128:(k+1)*128, mg*512:(mg+1)*512]
    xt = nc.dram_tensor("xt", [KPC * 4 * 128, 512], bf16, kind="ExternalInput").ap()
    wfc = nc.dram_tensor("wfc", [KPC * 128, D_FC], bf16, kind="ExternalInput").ap()
    oh_src = nc.dram_tensor("oh_src", [S, C], bf16, kind="ExternalInput").ap()
    o_own_src = nc.dram_tensor("o_own_src", [128, C], f32, kind="ExternalInput").ap()
    e_scaled = nc.dram_tensor("e_scaled", [OWN_T * 128, N], bf16, kind="ExternalInput").ap()
    bfc_row = nc.dram_tensor("bfc_row", [1, D_FC], f32, kind="ExternalInput").ap()
    bg1_row = nc.dram_tensor("bg1_row", [1, D_G1], f32, kind="ExternalInput").ap()
    bg2_row = nc.dram_tensor("bg2_row", [1, D_G2], f32, kind="ExternalInput").ap()
    bctr_in = nc.dram_tensor("bctr", [128, 4 * C], f32, kind="ExternalInput").ap()
    wg1_in = nc.dram_tensor("wg1", [D_G1, D_G1], bf16, kind="ExternalInput").ap()
    wg2_in = nc.dram_tensor("wg2", [D_G1, D_G2], bf16, kind="ExternalInput").ap()

    feats_sh = nc.dram_tensor("feats_sh", [OWN_T * 128, D_FC], f32, kind="ExternalOutput").ap()
    a_sh = nc.dram_tensor("a_sh", [OWN_T * 128, N], f32, kind="ExternalOutput").ap()
    h_sh = nc.dram_tensor("h_sh", [OWN_T * 128, D_G2], f32, kind="ExternalOutput").ap()

    RG = [list(range(NCORES))]
    AS = "Local" if single_core else "Shared"

    with tile.TileContext(nc) as tc:
        with (
            tc.tile_pool(name="dram", bufs=1, space="DRAM") as dram,
            tc.tile_pool(name="persist", bufs=1) as pp,
            tc.tile_pool(name="small", bufs=1) as sp,
            tc.tile_pool(name="ps_big", bufs=4, space="PSUM") as ps_big,
        ):
            for _rep in range(repeat):
                # collective bounce buffers (DRAM pool tiles => dep tracking)
                R = _rep
                pf_b = [dram.tile([S, D_FC], bf16, name=f"pf_b{hf}_{R}", tag=f"pfb{hf}_{R}")
                        for hf in range(2)]
                fo_d = [dram.tile([128, D_FC], bf16, name=f"fo_d{hf}_{R}", tag=f"fod{hf}_{R}")
                        for hf in range(2)]
                ct_bounce = dram.tile([128, 4 * C], f32, name=f"ct_bounce{R}", tag=f"ctb{R}")
                ct_ar = dram.tile([128, 4 * C], f32, addr_space=AS, name=f"ct_ar{R}", tag=f"cta{R}")
                og_in = dram.tile([OWN_T * 128, C], f32, name=f"og_in{R}", tag=f"ogi{R}")
                og_out = dram.tile([N, C], f32, addr_space=AS, name=f"og_out{R}", tag=f"ogo{R}")
                u1_in = dram.tile([C, 2 * D_G1 + 2], f32, name=f"u1_in{R}", tag=f"u1i{R}")
                u1_ar = dram.tile([C, 2 * D_G1 + 2], f32, addr_space=AS, name=f"u1_ar{R}", tag=f"u1a{R}")
                g2_in = dram.tile([C, 2 * D_G2], f32, name=f"g2_in{R}", tag=f"g2i{R}")
                g2_ar = dram.tile([C, 2 * D_G2], f32, addr_space=AS, name=f"g2_ar{R}", tag=f"g2a{R}")

                # ---------------- constants ----------------
                eye = pp.tile([128, 128], f32, tag="eye")
                make_identity(nc, eye[:])
                ones_col = pp.tile([128, 1], f32, tag="ones_col")
                nc.vector.memset(ones_col[:], 1.0)
                ones_row = pp.tile([1, 128], f32, tag="ones_row")
                nc.vector.memset(ones_row[:], 1.0)

                oh_all = pp.tile([128, 8 * C], bf16, tag="oh_all")
                nc.sync.dma_start(oh_all[:], oh_src.rearrange("(t p) c -> p t c", p=128))
                oh_t = [oh_all[:, t * C:(t + 1) * C] for t in range(8)]
                oso = pp.tile([128, C], f32, tag="oso")
                nc.sync.dma_start(oso[:], o_own_src[:])
                bctr = pp.tile([128, 4 * C], f32, tag="bctr")
                nc.sync.dma_start(bctr[:], bctr_in[:])

                # ---------------- phase 1: big FC matmul (bf16, K-sharded) --
                ct_sb = pp.tile([128, 4 * C], f32, tag="ct_sb")
                with (
                    tc.tile_pool(name="xt_pool", bufs=12) as xt_pool,
                    tc.tile_pool(name="w_pool", bufs=KPC) as w_pool,
                    tc.tile_pool(name="pf_pool", bufs=4) as pf_pool,
                    tc.tile_pool(name="ps_ct", bufs=1, space="PSUM") as ps_ct,
                ):
                    ps_c = ps_ct.tile([128, 4 * C], f32, tag="ps_c")
                    w_t = []
                    for k in range(KPC):
                        wk = w_pool.tile([128, D_FC], bf16, tag="w", name=f"w_{k}")
                        nc.sync.dma_start(wk[:], wfc[k * 128:(k + 1) * 128, :])
                        w_t.append(wk)

                    for mg in range(4):
                        pss = [ps_big.tile([128, D_FC], f32, tag="ps_mm", name=f"ps_mm{mi}")
                               for mi in range(4)]
                        for k in range(KPC):
                            xtb = xt_pool.tile([128, 512], bf16, tag="xt", name="xtb")
                            nc.sync.dma_start(xtb[:], xt[(k * 4 + mg) * 128:(k * 4 + mg + 1) * 128, :])
                            for mi in range(4):
                                nc.tensor.matmul(
                                    pss[mi][:], xtb[:, mi * 128:(mi + 1) * 128], w_t[k][:],
                                    start=(k == 0), stop=(k == KPC - 1))
                        for mi in range(4):
                            m = mg * 4 + mi
                            hf, mh = (0, m) if m < 8 else (1, m - 8)
                            pf = pf_pool.tile([128, D_FC], bf16, tag="pf", name="pf")
                            nc.vector.tensor_copy(pf[:], pss[mi][:])
                            nc.sync.dma_start(pf_b[hf][mh * 128:(mh + 1) * 128, :], pf[:])
                            if m < 8:
                                # partial centersT (bf16 lhsT; exact one-hot rhs)
                                for fc in range(4):
                                    nc.tensor.matmul(
                                        ps_c[:, fc * C:(fc + 1) * C],
                                        pf[:, fc * 128:(fc + 1) * 128], oh_t[m],
                                        start=(m == 0), stop=(m == 7))
                        if mg == 1:
                            # src half done: fire AR(centersT) + RS(src half)
                            nc.vector.tensor_copy(ct_sb[:], ps_c[:])
                            nc.sync.dma_start(ct_bounce[:], ct_sb[:])
                            if single_core:
                                nc.sync.dma_start(ct_ar[:], ct_bounce[:])
                                nc.sync.dma_start(fo_d[0][:], pf_b[0][0:128, :])
                            else:
                                nc.gpsimd.collective_compute(
                                    "AllReduce", Alu.add, replica_groups=RG,
                                    ins=[ct_bounce.opt()], outs=[ct_ar.opt()])
                                nc.gpsimd.collective_compute(
                                    "ReduceScatter", Alu.add, replica_groups=RG,
                                    ins=[pf_b[0].opt()], outs=[fo_d[0].opt()])
                    if single_core:
                        nc.sync.dma_start(fo_d[1][:], pf_b[1][0:128, :])
                    else:
                        nc.gpsimd.collective_compute(
                            "ReduceScatter", Alu.add, replica_groups=RG,
                            ins=[pf_b[1].opt()], outs=[fo_d[1].opt()])

                # ---------------- small persistent loads (overlap RS) -------
                brow_fc = pp.tile([1, D_FC], f32, tag="brow_fc")
                nc.sync.dma_start(brow_fc[:], bfc_row[:])
                brow_g1 = pp.tile([1, D_G1], f32, tag="brow_g1")
                nc.sync.dma_start(brow_g1[:], bg1_row[:])
                brow_g2 = pp.tile([1, D_G2], f32, tag="brow_g2")
                nc.sync.dma_start(brow_g2[:], bg2_row[:])

                def bcast_row(row, d, tag):
                    ps = ps_big.tile([128, d], f32, tag="ps_mm", name="ps_b")
                    nc.tensor.matmul(ps[:], ones_row[:], row[:], start=True, stop=True)
                    t = pp.tile([128, d], f32, tag=tag, name=tag)
                    nc.vector.tensor_copy(t[:], ps[:])
                    return t

                bb_fc = bcast_row(brow_fc, D_FC, "bb_fc")
                bb_g1 = bcast_row(brow_g1, D_G1, "bb_g1")
                bb_g2 = bcast_row(brow_g2, D_G2, "bb_g2")

                wg1 = [pp.tile([128, D_G1], bf16, tag=f"wg1_{k}", name=f"wg1_{k}") for k in range(4)]
                for k in range(4):
                    nc.sync.dma_start(wg1[k][:], wg1_in[k * 128:(k + 1) * 128, :])
                wg2 = [pp.tile([128, D_G2], bf16, tag=f"wg2_{k}", name=f"wg2_{k}") for k in range(4)]
                for k in range(4):
                    nc.sync.dma_start(wg2[k][:], wg2_in[k * 128:(k + 1) * 128, :])
                e_t = [pp.tile([128, N], bf16, tag=f"e_{t}", name=f"e_{t}") for t in range(OWN_T)]
                for t in range(OWN_T):
                    nc.sync.dma_start(e_t[t][:], e_scaled[t * 128:(t + 1) * 128, :])

                with (
                    tc.tile_pool(name="ps_tr", bufs=2, space="PSUM") as ps_tr,
                    tc.tile_pool(name="ps_wide", bufs=1, space="PSUM") as ps_wide,
                ):
                    # centersT back (+ bias correction: outer(b_fc, counts_src))
                    ct2 = pp.tile([128, 4 * C], f32, tag="ct2")
                    nc.sync.dma_start(ct2[:], ct_ar[:])
                    nc.vector.tensor_tensor(ct2[:], ct2[:], bctr[:], op=Alu.add)

                    # cnorm = -0.5 * sum_f center_c^2   -> [1, C]
                    sq = sp.tile([128, 4 * C], f32, tag="sq")
                    nc.vector.tensor_tensor(sq[:], ct2[:], ct2[:], op=Alu.mult)
                    ps_cn = ps_tr.tile([1, 4 * C], f32, tag="pst", name="ps_cn")
                    nc.tensor.matmul(ps_cn[:], ones_col[:], sq[:], start=True, stop=True)
                    cn16 = sp.tile([1, 4 * C], f32, tag="cn16")
                    nc.vector.tensor_copy(cn16[:], ps_cn[:])
                    cnA = sp.tile([1, C], f32, tag="cnA")
                    nc.vector.tensor_tensor(cnA[:], cn16[:, 0:C], cn16[:, C:2 * C], op=Alu.add)
                    cnB = sp.tile([1, C], f32, tag="cnB")
                    nc.vector.tensor_tensor(cnB[:], cn16[:, 2 * C:3 * C], cn16[:, 3 * C:4 * C], op=Alu.add)
                    neghcn = sp.tile([1, C], f32, tag="neghcn")
                    nc.vector.tensor_tensor(neghcn[:], cnA[:], cnB[:], op=Alu.add)
                    nc.vector.tensor_scalar(neghcn[:], neghcn[:], -0.5, None, op0=Alu.mult)

                    # feats_own (+bias): tile0 = src half rows, tile1 = tgt half
                    feats_own = [pp.tile([128, D_FC], f32, tag=f"feats_own_{t}",
                                         name=f"feats_own_{t}") for t in range(OWN_T)]
                    for t in range(OWN_T):
                        fbf = sp.tile([128, D_FC], bf16, tag="fbf", bufs=2, name="fbf")
                        nc.sync.dma_start(fbf[:], fo_d[t][:])
                        nc.vector.tensor_tensor(feats_own[t][:], fbf[:], bb_fc[:], op=Alu.add)
                        nc.sync.dma_start(feats_sh[t * 128:(t + 1) * 128, :], feats_own[t][:])

                    # transposed feats (bf16) for X1 matmuls / scores
                    ftT = [pp.tile([128, 128], bf16, tag=f"ftT_{i}", name=f"ftT_{i}")
                           for i in range(OWN_T * 4)]
                    for t in range(OWN_T):
                        for fc in range(4):
                            pst = ps_tr.tile([128, 128], f32, tag="pst", name="ps_t")
                            nc.tensor.transpose(pst[:], feats_own[t][:, fc * 128:(fc + 1) * 128], eye[:])
                            nc.vector.tensor_copy(ftT[t * 4 + fc][:], pst[:])

                    # bf16 copy of centersT for the scores matmul
                    ct2b = sp.tile([128, 4 * C], bf16, tag="ct2b")
                    nc.vector.tensor_copy(ct2b[:], ct2[:])

                    # scores -> target one-hot mask (tile 1 only)
                    ps_s = ps_tr.tile([128, C], f32, tag="pst", name="ps_s")
                    for fc in range(4):
                        nc.tensor.matmul(ps_s[:], ftT[4 + fc][:],
                                         ct2b[:, fc * C:(fc + 1) * C],
                                         start=(fc == 0), stop=False)
                    nc.tensor.matmul(ps_s[:], ones_row[:], neghcn[:], start=False, stop=True)
                    mx = sp.tile([128, 1], f32, tag="mx")
                    nc.vector.tensor_reduce(mx[:], ps_s[:], axis=mybir.AxisListType.X,
                                            op=Alu.max)
                    mask = pp.tile([128, C], f32, tag="mask")
                    nc.vector.tensor_scalar(mask[:], ps_s[:], mx[:], None, op0=Alu.is_ge)

                    o_own = [oso, mask]
                    for t in range(OWN_T):
                        nc.sync.dma_start(og_in[t * 128:(t + 1) * 128, :], o_own[t][:])

                    # X1 = feats_own @ W_g1 (unscaled; dinv applied post-AR)
                    x1_sb = []
                    for t in range(OWN_T):
                        ps_x = ps_big.tile([128, D_G1], f32, tag="ps_mm", name="ps_x1")
                        for fc in range(4):
                            nc.tensor.matmul(ps_x[:], ftT[t * 4 + fc][:], wg1[fc][:],
                                             start=(fc == 0), stop=(fc == 3))
                        xs = pp.tile([128, D_G1], f32, tag=f"x1_{t}", name=f"x1_{t}")
                        nc.vector.tensor_copy(xs[:], ps_x[:])
                        x1_sb.append(xs)

                    # U1 = [sum_src-class X1 | sum_tgt-class X1]  (no dinv yet)
                    ps_u1 = ps_wide.tile([C, 2 * D_G1], f32, tag="ps_g", name="ps_u1")
                    nc.tensor.matmul(ps_u1[:, 0:D_G1], oso[:], x1_sb[0][:],
                                     start=True, stop=True)
                    nc.tensor.matmul(ps_u1[:, D_G1:2 * D_G1], mask[:], x1_sb[1][:],
                                     start=True, stop=True)
                    # own class counts ride along with U1
                    ps_co = ps_tr.tile([C, 2], f32, tag="pst", name="ps_co")
                    nc.tensor.matmul(ps_co[:, 0:1], oso[:], ones_col[:], start=True, stop=True)
                    nc.tensor.matmul(ps_co[:, 1:2], mask[:], ones_col[:], start=True, stop=True)
                    u1_sb = sp.tile([C, 2 * D_G1 + 2], f32, tag="u1_sb")
                    nc.vector.tensor_copy(u1_sb[:, 0:2 * D_G1], ps_u1[:])
                    nc.vector.tensor_copy(u1_sb[:, 2 * D_G1:2 * D_G1 + 2], ps_co[:])
                    nc.sync.dma_start(u1_in[:], u1_sb[:])

                    # collectives: AG(one-hots) then AR(U1), back-to-back
                    if single_core:
                        for _c in range(NCORES):
                            nc.sync.dma_start(og_out[_c * 256:(_c + 1) * 256, :], og_in[:])
                        nc.sync.dma_start(u1_ar[:], u1_in[:])
                    else:
                        nc.gpsimd.collective_compute(
                            "AllGather", Alu.bypass, replica_groups=RG,
                            ins=[og_in.opt()], outs=[og_out.opt()])
                        nc.gpsimd.collective_compute(
                            "AllReduce", Alu.add, replica_groups=RG,
                            ins=[u1_in.opt()], outs=[u1_ar.opt()])

                    # O_full in global node order, batched as 2 DMAs:
                    # src tiles t: og_out rows 256t+p; tgt tiles: rows 256t+128+p
                    of_all = [pp.tile([128, 8 * C], f32, tag=f"of_all{hf}", name=f"of_all{hf}")
                              for hf in range(2)]
                    og_r = og_out.rearrange("(t q p) c -> q p t c", q=2, p=128)
                    nc.sync.dma_start(of_all[0][:], og_r[0])
                    nc.sync.dma_start(of_all[1][:], og_r[1])
                    o_full = [of_all[t // 8][:, (t % 8) * C:(t % 8 + 1) * C] for t in range(NT)]

                    # counts arrive with the U1 AllReduce
                    u1r = sp.tile([C, 2 * D_G1 + 2], f32, tag="u1r")
                    nc.sync.dma_start(u1r[:], u1_ar[:])
                    cnt = u1r[:, 2 * D_G1:2 * D_G1 + 2]

                    deg2 = sp.tile([C, 2], f32, tag="deg2")  # col0 src rows, col1 tgt
                    nc.vector.tensor_scalar(deg2[:, 0:1], cnt[:, 1:2], 2.0, None, op0=Alu.add)
                    nc.vector.tensor_tensor(deg2[:, 1:2], cnt[:, 0:1], cnt[:, 1:2], op=Alu.add)
                    nc.vector.tensor_scalar(deg2[:, 1:2], deg2[:, 1:2], 1.0, None, op0=Alu.add)

                    # dinv = rsqrt(deg): seed sqrt(1/deg) + 2 Newton steps
                    rec = sp.tile([C, 2], f32, tag="rec")
                    nc.vector.reciprocal(rec[:], deg2[:])
                    dinv2 = sp.tile([C, 2], f32, tag="dinv2")
                    nc.scalar.activation(dinv2[:], rec[:], mybir.ActivationFunctionType.Sqrt)
                    tmp_a = sp.tile([C, 2], f32, tag="tmp_a")
                    tmp_b = sp.tile([C, 2], f32, tag="tmp_b")
                    for _ in range(1):
                        nc.vector.tensor_tensor(tmp_a[:], dinv2[:], dinv2[:], op=Alu.mult)
                        nc.vector.tensor_tensor(tmp_a[:], tmp_a[:], deg2[:], op=Alu.mult)
                        nc.vector.tensor_scalar(tmp_b[:], tmp_a[:], -0.5, 1.5,
                                                op0=Alu.mult, op1=Alu.add)
                        nc.vector.tensor_tensor(dinv2[:], dinv2[:], tmp_b[:], op=Alu.mult)
                    dinvsq2 = sp.tile([C, 2], f32, tag="dinvsq2")
                    nc.vector.tensor_tensor(dinvsq2[:], dinv2[:], dinv2[:], op=Alu.mult)

                    # ---------------- P matrix [4, 2048] ----------------
                    p_sb = pp.tile([C, N], f32, tag="p_sb")
                    for t in range(NT):
                        ps_o = ps_tr.tile([C, 128], f32, tag="pst", name="ps_o")
                        nc.tensor.transpose(ps_o[:], o_full[t], eye[:])
                        otT = sp.tile([C, 128], f32, tag="otT", bufs=2)
                        nc.vector.tensor_copy(otT[:], ps_o[:])
                        col = 0 if t < 8 else 1
                        ps_d = ps_tr.tile([128, 1], f32, tag="pst", name="ps_d")
                        nc.tensor.matmul(ps_d[:], otT[:], dinv2[:, col:col + 1],
                                         start=True, stop=True)
                        q = sp.tile([128, C], f32, tag="q", bufs=2)
                        nc.vector.tensor_scalar(q[:], o_full[t], ps_d[:], None,
                                                op0=Alu.mult)
                        ps_q = ps_tr.tile([C, 128], f32, tag="pst", name="ps_q")
                        nc.tensor.transpose(ps_q[:], q[:], eye[:])
                        nc.vector.tensor_copy(p_sb[:, t * 128:(t + 1) * 128], ps_q[:])

                    # own-row quantities (tile0: src type col=0, tile1: tgt col=1)
                    otT_own, pown, dinv_own, dinvsq_own = [], [], [], []
                    for t in range(OWN_T):
                        ps_o = ps_tr.tile([C, 128], f32, tag="pst", name="ps_o2")
                        nc.tensor.transpose(ps_o[:], o_own[t][:], eye[:])
                        ot = pp.tile([C, 128], f32, tag=f"otT_own_{t}", name=f"otT_own_{t}")
                        nc.vector.tensor_copy(ot[:], ps_o[:])
                        otT_own.append(ot)

                        col = t  # tile0 -> src col, tile1 -> tgt col
                        ps_d = ps_tr.tile([128, 2], f32, tag="pst", name="ps_d2")
                        nc.tensor.matmul(ps_d[:, 0:1], ot[:], dinv2[:, col:col + 1],
                                         start=True, stop=True)
                        nc.tensor.matmul(ps_d[:, 1:2], ot[:], dinvsq2[:, col:col + 1],
                                         start=True, stop=True)
                        dv = pp.tile([128, 2], f32, tag=f"dinv_own_{t}", name=f"dinv_own_{t}")
                        nc.vector.tensor_copy(dv[:], ps_d[:])
                        dinv_own.append(dv[:, 0:1])
                        dinvsq_own.append(dv[:, 1:2])

                        q = sp.tile([128, C], f32, tag="q_own")
                        nc.vector.tensor_scalar(q[:], o_own[t][:], dv[:, 0:1], None,
                                                op0=Alu.mult)
                        ps_q = ps_tr.tile([C, 128], f32, tag="pst", name="ps_q2")
                        nc.tensor.transpose(ps_q[:], q[:], eye[:])
                        po = pp.tile([C, 128], f32, tag=f"pown_{t}", name=f"pown_{t}")
                        nc.vector.tensor_copy(po[:], ps_q[:])
                        pown.append(po)

                    # ---------------- A_norm rows ----------------
                    # tile0 (src rows): cols < 1024 diag-only; cols >= 1024 dense
                    for j in range(2):
                        a_sb = sp.tile([128, 512], f32, tag="a_sb", bufs=2)
                        nc.vector.tensor_scalar(a_sb[:], e_t[0][:, j * 512:(j + 1) * 512],
                                                dinvsq_own[0], None, op0=Alu.mult)
                        nc.sync.dma_start(a_sh[0:128, j * 512:(j + 1) * 512], a_sb[:])
                    for j in range(2, 4):
                        ps_a = ps_big.tile([128, 512], f32, tag="ps_mm", name="ps_a")
                        nc.tensor.matmul(ps_a[:], pown[0][:], p_sb[:, j * 512:(j + 1) * 512],
                                         start=True, stop=True)
                        a_sb = sp.tile([128, 512], f32, tag="a_sb", bufs=2)
                        nc.vector.scalar_tensor_tensor(
                            a_sb[:], e_t[0][:, j * 512:(j + 1) * 512], dinvsq_own[0],
                            ps_a[:], op0=Alu.mult, op1=Alu.add)
                        nc.sync.dma_start(a_sh[0:128, j * 512:(j + 1) * 512], a_sb[:])
                    for j in range(4):
                        ps_a = ps_big.tile([128, 512], f32, tag="ps_mm", name="ps_a2")
                        nc.tensor.matmul(ps_a[:], pown[1][:], p_sb[:, j * 512:(j + 1) * 512],
                                         start=True, stop=True)
                        a_sb = sp.tile([128, 512], f32, tag="a_sb", bufs=2)
                        nc.vector.scalar_tensor_tensor(
                            a_sb[:], e_t[1][:, j * 512:(j + 1) * 512], dinvsq_own[1],
                            ps_a[:], op0=Alu.mult, op1=Alu.add)
                        nc.sync.dma_start(a_sh[128:256, j * 512:(j + 1) * 512], a_sb[:])

                    # ---------------- GCN layer 1 (post-AR) ----------------
                    # G_src = dinv_src_class * U_src ; G_tgt = dinv_tgt_class * U_tgt
                    g1_tgt = sp.tile([C, D_G1], f32, tag="g1_tgt")
                    nc.vector.tensor_scalar(g1_tgt[:], u1r[:, D_G1:2 * D_G1],
                                            dinv2[:, 1:2], None, op0=Alu.mult)
                    g1_all = sp.tile([C, D_G1], f32, tag="g1_all")
                    nc.vector.scalar_tensor_tensor(g1_all[:], u1r[:, 0:D_G1],
                                                   dinv2[:, 0:1], g1_tgt[:],
                                                   op0=Alu.mult, op1=Alu.add)
                    g1_use = [g1_tgt, g1_all]
                    c_own = [2.0, 1.0]

                    h1 = []
                    for t in range(OWN_T):
                        yt = sp.tile([128, D_G1], f32, tag=f"y1_{t}", name=f"y1_{t}")
                        nc.vector.tensor_scalar(yt[:], x1_sb[t][:], dinv_own[t], None,
                                                op0=Alu.mult)
                        ps_h = ps_big.tile([128, D_G1], f32, tag="ps_mm", name="ps_h1")
                        nc.tensor.matmul(ps_h[:], otT_own[t][:], g1_use[t][:],
                                         start=True, stop=True)
                        u = sp.tile([128, D_G1], f32, tag="u1t", bufs=2)
                        nc.vector.scalar_tensor_tensor(u[:], yt[:], c_own[t], ps_h[:],
                                                       op0=Alu.mult, op1=Alu.add)
                        ht = pp.tile([128, D_G1], f32, tag=f"h1_{t}", name=f"h1_{t}")
                        nc.vector.scalar_tensor_tensor(ht[:], u[:], dinv_own[t], bb_g1[:],
                                                       op0=Alu.mult, op1=Alu.add)
                        nc.vector.tensor_scalar(ht[:], ht[:], 0.0, None, op0=Alu.max)
                        h1.append(ht)

                    # ---------------- GCN layer 2 ----------------
                    y2 = []
                    for t in range(OWN_T):
                        h1T = []
                        for fc in range(4):
                            pst = ps_tr.tile([128, 128], f32, tag="pst", name="ps_t2")
                            nc.tensor.transpose(pst[:], h1[t][:, fc * 128:(fc + 1) * 128], eye[:])
                            hT = sp.tile([128, 128], bf16, tag=f"h1T_{fc}", bufs=2,
                                         name=f"h1T_{fc}")
                            nc.vector.tensor_copy(hT[:], pst[:])
                            h1T.append(hT)
                        ps_x = ps_big.tile([128, D_G2], f32, tag="ps_mm", name="ps_x2")
                        for fc in range(4):
                            nc.tensor.matmul(ps_x[:], h1T[fc][:], wg2[fc][:],
                                             start=(fc == 0), stop=(fc == 3))
                        yt = pp.tile([128, D_G2], f32, tag=f"y2_{t}", name=f"y2_{t}")
                        nc.vector.tensor_scalar(yt[:], ps_x[:], dinv_own[t], None, op0=Alu.mult)
                        y2.append(yt)

                    ps_g2 = ps_wide.tile([C, 2 * D_G2], f32, tag="ps_g", name="ps_g2")
                    nc.tensor.matmul(ps_g2[:, 0:D_G2], oso[:], y2[0][:], start=True, stop=True)
                    nc.tensor.matmul(ps_g2[:, D_G2:2 * D_G2], mask[:], y2[1][:],
                                     start=True, stop=True)
                    g2_sb = sp.tile([C, 2 * D_G2], f32, tag="g2_sb")
                    nc.vector.tensor_copy(g2_sb[:], ps_g2[:])
                    nc.sync.dma_start(g2_in[:], g2_sb[:])
                    if single_core:
                        nc.sync.dma_start(g2_ar[:], g2_in[:])
                    else:
                        nc.gpsimd.collective_compute(
                            "AllReduce", Alu.add, replica_groups=RG,
                            ins=[g2_in.opt()], outs=[g2_ar.opt()])
                    g2r = sp.tile([C, 2 * D_G2], f32, tag="g2r")
                    nc.sync.dma_start(g2r[:], g2_ar[:])
                    g2_tgt = sp.tile([C, D_G2], f32, tag="g2_tgt")
                    nc.vector.tensor_copy(g2_tgt[:], g2r[:, D_G2:2 * D_G2])
                    g2_all = sp.tile([C, D_G2], f32, tag="g2_all")
                    nc.vector.tensor_tensor(g2_all[:], g2r[:, 0:D_G2], g2_tgt[:], op=Alu.add)
                    g2_use = [g2_tgt, g2_all]

                    for t in range(OWN_T):
                        ps_h = ps_big.tile([128, D_G2], f32, tag="ps_mm", name="ps_h2")
                        nc.tensor.matmul(ps_h[:], otT_own[t][:], g2_use[t][:],
                                         start=True, stop=True)
                        u = sp.tile([128, D_G2], f32, tag="u2t", bufs=2)
                        nc.vector.scalar_tensor_tensor(u[:], y2[t][:], c_own[t], ps_h[:],
                                                       op0=Alu.mult, op1=Alu.add)
                        hh = sp.tile([128, D_G2], f32, tag="hh", bufs=2)
                        nc.vector.scalar_tensor_tensor(hh[:], u[:], dinv_own[t], bb_g2[:],
                                                       op0=Alu.mult, op1=Alu.add)
                        nc.vector.tensor_scalar(hh[:], hh[:], 0.0, None, op0=Alu.max)
                        nc.sync.dma_start(h_sh[t * 128:(t + 1) * 128, :], hh[:])

    nc.compile()
    _PROGRAM_CACHE[key] = nc
    return nc


def _host_inputs(x, W_fc, b_fc, W_g1, b_g1, W_g2, b_g2, source_labels):
    """Build per-core in_maps."""
    x = np.asarray(x, dtype=np.float32)
    W_fc = np.asarray(W_fc, dtype=np.float32)
    b_fc = np.asarray(b_fc, dtype=np.float32)
    labels = np.asarray(source_labels).astype(np.int64)

    x_bf = x.astype(BF16)
    w_bf = W_fc.astype(BF16)
    wg1_bf = np.asarray(W_g1, np.float32).astype(BF16)
    wg2_bf = np.asarray(W_g2, np.float32).astype(BF16)

    onehot_src = np.zeros((S, C), np.float32)
    onehot_src[np.arange(S), labels] = 1.0
    counts_src = onehot_src.sum(axis=0)

    # bias correction for centers: centersT chunks bctr[128, 4*C]
    bctr = np.zeros((128, 4 * C), np.float32)
    for fc in range(4):
        bctr[:, fc * C:(fc + 1) * C] = np.outer(b_fc[fc * 128:(fc + 1) * 128],
                                                counts_src)

    offs = np.cumsum([0] + KSPLIT)
    in_maps = []
    r = np.arange(128)
    for i in range(NCORES):
        c0 = offs[i] * 128
        nk = offs[i + 1] - offs[i]
        c1 = offs[i + 1] * 128
        xt_i = np.zeros((KPC, 4, 128, 512), BF16)
        xT = x_bf[:, c0:c1].T  # [nk*128, 2048]
        xt_i[:nk] = xT.reshape(nk, 128, 4, 512).transpose(0, 2, 1, 3)
        xt_i = np.ascontiguousarray(xt_i.reshape(KPC * 4 * 128, 512))
        wf_i = np.zeros((KPC * 128, D_FC), BF16)
        wf_i[:nk * 128] = w_bf[c0:c1]

        # own rows: src nodes [128i, 128i+128), tgt nodes [1024+128i, ...)
        e_sc = np.zeros((256, N), BF16)
        e_sc[r, 128 * i + r] = BF16(2.0)
        e_sc[128 + r, 1024 + 128 * i + r] = BF16(1.0)

        in_maps.append({
            "xt": xt_i,
            "wfc": wf_i,
            "oh_src": onehot_src.astype(BF16),
            "o_own_src": np.ascontiguousarray(onehot_src[128 * i:128 * (i + 1)]),
            "e_scaled": e_sc,
            "bfc_row": b_fc.reshape(1, D_FC),
            "bg1_row": np.asarray(b_g1, np.float32).reshape(1, D_G1),
            "bg2_row": np.asarray(b_g2, np.float32).reshape(1, D_G2),
            "bctr": bctr,
            "wg1": wg1_bf,
            "wg2": wg2_bf,
        })
    return in_maps


def _assemble(parts):
    """parts[i]: [256, d] rows = (src nodes 128i.., tgt nodes 1024+128i..)."""
    d = parts[0].shape[1]
    full = np.empty((N, d), np.float32)
    for i in range(NCORES):
        full[128 * i:128 * (i + 1)] = parts[i][0:128]
        full[1024 + 128 * i:1024 + 128 * (i + 1)] = parts[i][128:256]
    return full


def kernel(x, W_fc, b_fc, W_g1, b_g1, W_g2, b_g2, source_labels, source_length):
    from concourse import bass_utils

    assert int(source_length) == S
    nc = _build_program()
    in_maps = _host_inputs(x, W_fc, b_fc, W_g1, b_g1, W_g2, b_g2, source_labels)
    res = bass_utils.run_bass_kernel_spmd(nc, in_maps, list(range(NCORES)))
    h = _assemble([np.asarray(res.results[i]["h_sh"]) for i in range(NCORES)])
    a = _assemble([np.asarray(res.results[i]["a_sh"]) for i in range(NCORES)])
    f = _assemble([np.asarray(res.results[i]["feats_sh"]) for i in range(NCORES)])
    return (h, a, f)
